# revision 10
# baseline (speedup 1.0000x reference)
"""CrossAttentionFusion forward on 8 Trainium2 NeuronCores (pure data parallel).

Math folded on host (seq-len-1 MHA == two chained linears):
  d_att = micro @ A_dm + c_dm,  A_dm = Wv_dm.T @ Wout_dm.T
  m_att = drug  @ A_md + c_md
  u = drug + d_att ; w = micro + m_att
  xu = (u - mu)/sd ; xw likewise        (LN affine folded into W1)
  h1 = gelu([xu, xw] @ W1f + b1f),  W1f = (ffn_w1 * g_cat).T
  h2 = h1 @ W2f + b2,               W2f = ffn_w2.T
  out = ((h2 - mu)/sd) * g_out + b_out

Device layout: activations feature-major [feat(partition), batch(free)];
batch sharded across 8 cores, tiles of NB=512 columns. LN stats via
M=2-packed ones-matmuls (sum and sumsq of both streams into two PSUM
banks); mu/rstd broadcast across partitions on the GpSimd engine
(partition_broadcast), keeping the tensor engine free of tiny matmuls.
The per-tile work is software-pipelined 3 deep so the tensor engine
never waits on the LN stats chain. All matmuls bf16 with fp32 PSUM.
"""

import sys

if "/opt/trn_rl_repo" not in sys.path:
    sys.path.insert(0, "/opt/trn_rl_repo")

from contextlib import ExitStack

import ml_dtypes
import numpy as np

import concourse.bass as bass  # noqa: F401  (registers mybir lowering hooks)
import concourse.tile as tile
from concourse import bacc, mybir
from concourse.bass import ts
from concourse.bass_utils import run_bass_kernel_spmd

F32 = mybir.dt.float32
BF16 = mybir.dt.bfloat16
ACT = mybir.ActivationFunctionType

P = 128
D = 384
KD = D // P          # 3
DH = 2 * D           # 768
KH = DH // P         # 6
DF = 4 * D           # 1536
KF = DF // P         # 12
EPS = 1e-5
N_CORES = 8
B_FULL = 65536
BC = B_FULL // N_CORES   # 8192 rows per core
NB = 512                 # batch columns per on-chip tile
SQD = float(1.0 / np.sqrt(D))

_NC_CACHE = {}
LAST_RESULTS = None      # BassKernelResults of the most recent kernel() call


def _build_nc(bc, nb, flags):
    use_c_dm, use_c_md, use_b1, use_b2, use_affine = flags
    nt = bc // nb
    assert nt >= 4
    nc = bacc.Bacc("TRN2", target_bir_lowering=False, debug=False,
                   num_devices=N_CORES)

    xd_d = nc.dram_tensor("xd", [D, bc], BF16, kind="ExternalInput")
    xm_d = nc.dram_tensor("xm", [D, bc], BF16, kind="ExternalInput")
    a_dm_d = nc.dram_tensor("a_dm", [D, D], BF16, kind="ExternalInput")
    a_md_d = nc.dram_tensor("a_md", [D, D], BF16, kind="ExternalInput")
    w1_d = nc.dram_tensor("w1", [DH, DF], BF16, kind="ExternalInput")
    w2_d = nc.dram_tensor("w2", [DF, D], BF16, kind="ExternalInput")
    c_dm_d = nc.dram_tensor("c_dm", [D], F32, kind="ExternalInput") if use_c_dm else None
    c_md_d = nc.dram_tensor("c_md", [D], F32, kind="ExternalInput") if use_c_md else None
    b1_d = nc.dram_tensor("b1", [DF], F32, kind="ExternalInput") if use_b1 else None
    b2_d = nc.dram_tensor("b2", [D], F32, kind="ExternalInput") if use_b2 else None
    g_o_d = nc.dram_tensor("g_o", [D], F32, kind="ExternalInput") if use_affine else None
    b_o_d = nc.dram_tensor("b_o", [D], F32, kind="ExternalInput") if use_affine else None
    o_d = nc.dram_tensor("o", [D, bc], F32, kind="ExternalOutput")

    xd_r = xd_d.ap().rearrange("(k p) n -> p k n", p=P)
    xm_r = xm_d.ap().rearrange("(k p) n -> p k n", p=P)
    o_r = o_d.ap().rearrange("(k p) n -> p k n", p=P)

    with tile.TileContext(nc) as tc, ExitStack() as ctx:
        wp = ctx.enter_context(tc.tile_pool(name="wts", bufs=1))
        xp = ctx.enter_context(tc.tile_pool(name="x", bufs=3))
        up = ctx.enter_context(tc.tile_pool(name="u", bufs=3))
        sqp = ctx.enter_context(tc.tile_pool(name="sq", bufs=2))
        xhp = ctx.enter_context(tc.tile_pool(name="xh", bufs=3))
        h1p = ctx.enter_context(tc.tile_pool(name="h1", bufs=2))
        h2p = ctx.enter_context(tc.tile_pool(name="h2", bufs=3))
        sqo = ctx.enter_context(tc.tile_pool(name="sqo", bufs=2))
        op_ = ctx.enter_context(tc.tile_pool(name="o", bufs=2))
        stp = ctx.enter_context(tc.tile_pool(name="st", bufs=1))
        bcp = ctx.enter_context(tc.tile_pool(name="bc", bufs=4))
        obcp = ctx.enter_context(tc.tile_pool(name="obc", bufs=2))
        pmm = ctx.enter_context(tc.tile_pool(name="pmm", bufs=4, space="PSUM"))
        pstat = ctx.enter_context(tc.tile_pool(name="pstat", bufs=1, space="PSUM"))

        a_dm_sb = wp.tile([P, KD, D], BF16)
        nc.gpsimd.dma_start(a_dm_sb[:], a_dm_d.ap().rearrange("(k p) m -> p k m", p=P))
        a_md_sb = wp.tile([P, KD, D], BF16)
        nc.gpsimd.dma_start(a_md_sb[:], a_md_d.ap().rearrange("(k p) m -> p k m", p=P))
        w1_sb = wp.tile([P, KH, DF], BF16)
        nc.gpsimd.dma_start(w1_sb[:], w1_d.ap().rearrange("(k p) m -> p k m", p=P))
        w2_sb = wp.tile([P, KF, D], BF16)
        nc.gpsimd.dma_start(w2_sb[:], w2_d.ap().rearrange("(k p) m -> p k m", p=P))

        # lhsT constants for the packed stat sums:
        #   cwm[:, s, :] : [P, 2] column s = 1/D      -> mean rows
        #   cws[:, s, :] : [P, 2] column s = 1.0      -> sumsq rows
        cwm = wp.tile([P, 2, 2], BF16)
        nc.vector.memset(cwm[:], 0.0)
        nc.vector.memset(cwm[:, 0, 0:1], 1.0 / D)
        nc.vector.memset(cwm[:, 1, 1:2], 1.0 / D)
        cws = wp.tile([P, 2, 2], BF16)
        nc.vector.memset(cws[:], 0.0)
        nc.vector.memset(cws[:, 0, 0:1], 1.0)
        nc.vector.memset(cws[:, 1, 1:2], 1.0)
        eps2 = wp.tile([2, 1], F32)
        nc.vector.memset(eps2[:], EPS)

        def vec_const(dram, nk, tag):
            t = wp.tile([P, nk], F32, tag=tag)
            nc.gpsimd.dma_start(t[:], dram.ap().rearrange("(k p) -> p k", p=P))
            return t

        c_dm_sb = vec_const(c_dm_d, KD, "c_dm") if use_c_dm else None
        c_md_sb = vec_const(c_md_d, KD, "c_md") if use_c_md else None
        b1_sb = vec_const(b1_d, KF, "b1") if use_b1 else None
        b2_sb = vec_const(b2_d, KD, "b2") if use_b2 else None
        g_o_sb = vec_const(g_o_d, KD, "g_o") if use_affine else None
        b_o_sb = vec_const(b_o_d, KD, "b_o") if use_affine else None

        st = [dict() for _ in range(nt)]

        def load(i):
            s = st[i]
            sl = slice(i * nb, (i + 1) * nb)
            s["xd"] = xp.tile([P, KD, nb], BF16, tag="xd", name="xdt")
            nc.sync.dma_start(s["xd"][:], xd_r[:, :, sl])
            s["xm"] = xp.tile([P, KD, nb], BF16, tag="xm", name="xmt")
            nc.sync.dma_start(s["xm"][:], xm_r[:, :, sl])

        def attn(i):
            s = st[i]

            def side(a_sb, rhs, res, c_sb, tag):
                v = up.tile([P, KD, nb], BF16, tag=tag)
                for m in range(KD):
                    ps = pmm.tile([P, nb], F32, tag="mm")
                    for k in range(KD):
                        nc.tensor.matmul(ps[:], a_sb[:, k, ts(m, P)], rhs[:, k, :],
                                         start=(k == 0), stop=(k == KD - 1))
                    nc.vector.tensor_add(v[:, m, :], ps[:], res[:, m, :])
                    if c_sb is not None:
                        nc.vector.tensor_scalar_add(v[:, m, :], v[:, m, :],
                                                    c_sb[:, m:m + 1])
                return v

            s["u"] = side(a_dm_sb, s["xm"], s["xd"], c_dm_sb, "u")
            s["w"] = side(a_md_sb, s["xd"], s["xm"], c_md_sb, "w")

        def squares(i):
            s = st[i]
            s["squ"] = sqp.tile([P, KD, nb], BF16, tag="squ", name="squ")
            s["sqw"] = sqp.tile([P, KD, nb], BF16, tag="sqw", name="sqw")
            for k in range(KD):
                nc.scalar.activation(s["squ"][:, k, :], s["u"][:, k, :],
                                     ACT.Square, scale=SQD)
                nc.scalar.activation(s["sqw"][:, k, :], s["w"][:, k, :],
                                     ACT.Square, scale=SQD)

        def sums(i):
            """ps_mu rows [mu_u, mu_w]; ps_sq rows [E[u^2], E[w^2]]."""
            s = st[i]
            ps_mu = pstat.tile([2, nb], F32, tag="smu")
            n = 0
            for sdx, x in ((0, s["u"]), (1, s["w"])):
                for k in range(KD):
                    nc.tensor.matmul(ps_mu[:], cwm[:, sdx, :], x[:, k, :],
                                     start=(n == 0), stop=(n == 2 * KD - 1))
                    n += 1
            ps_sq = pstat.tile([2, nb], F32, tag="ssq")
            n = 0
            for sdx, x in ((0, s["squ"]), (1, s["sqw"])):
                for k in range(KD):
                    nc.tensor.matmul(ps_sq[:], cws[:, sdx, :], x[:, k, :],
                                     start=(n == 0), stop=(n == 2 * KD - 1))
                    n += 1
            s["ps_mu"], s["ps_sq"] = ps_mu, ps_sq

        def chain(i):
            """stats chain on partitions 0..1; broadcast via GpSimd."""
            s = st[i]
            ps_mu, ps_sq = s.pop("ps_mu"), s.pop("ps_sq")
            mu_sb = stp.tile([2, nb], F32, tag="mu_sb")
            nc.vector.tensor_copy(mu_sb[:], ps_mu[:])
            tmp = stp.tile([2, nb], F32, tag="tmp")
            nc.vector.tensor_mul(tmp[:], mu_sb[:], mu_sb[:])
            nc.vector.tensor_sub(tmp[:], ps_sq[:], tmp[:])
            nc.scalar.activation(tmp[:], tmp[:], ACT.Sqrt, bias=eps2[:])
            inv = stp.tile([2, nb], F32, tag="inv")
            nc.vector.reciprocal_approx_fast(inv[:], tmp[:])
            s2t = stp.tile([2, 2, nb], BF16, tag="s2t")
            nc.vector.tensor_copy(s2t[:, 0, :], mu_sb[:])
            nc.vector.tensor_copy(s2t[:, 1, :], inv[:])
            # GpSimd ops must start at partition 0; DMA row 1 down first.
            s2tw = stp.tile([1, 2, nb], BF16, tag="s2tw")
            nc.sync.dma_start(s2tw[:], s2t[1:2, :, :])
            s["bcu"] = bcp.tile([P, 2, nb], BF16, tag="bc", name="bcu")
            nc.gpsimd.partition_broadcast(s["bcu"][:], s2t[0:1, :, :])
            s["bcw"] = bcp.tile([P, 2, nb], BF16, tag="bc", name="bcw")
            nc.gpsimd.partition_broadcast(s["bcw"][:], s2tw[:])

        def applies(i):
            s = st[i]
            s["xhu"] = xhp.tile([P, KD, nb], BF16, tag="xhu", name="xhu")
            s["xhw"] = xhp.tile([P, KD, nb], BF16, tag="xhw", name="xhw")
            for x, bc, xh in ((s["u"], s["bcu"], s["xhu"]),
                              (s["w"], s["bcw"], s["xhw"])):
                for k in range(KD):
                    nc.vector.tensor_sub(xh[:, k, :], x[:, k, :], bc[:, 0, :])
                    nc.vector.tensor_mul(xh[:, k, :], xh[:, k, :], bc[:, 1, :])

        def ffn1(i):
            s = st[i]
            xhu, xhw = s["xhu"], s["xhw"]
            h1 = h1p.tile([P, KF, nb], BF16, tag="h1")
            for m in range(KF):
                ps = pmm.tile([P, nb], F32, tag="mm")
                for k in range(KH):
                    rhs = xhu[:, k, :] if k < KD else xhw[:, k - KD, :]
                    nc.tensor.matmul(ps[:], w1_sb[:, k, ts(m, P)], rhs,
                                     start=(k == 0), stop=(k == KH - 1))
                if use_b1:
                    nc.scalar.activation(h1[:, m, :], ps[:], ACT.Gelu,
                                         bias=b1_sb[:, m:m + 1])
                else:
                    nc.scalar.activation(h1[:, m, :], ps[:], ACT.Gelu)
            s["h1"] = h1

        def ffn2(i):
            s = st[i]
            h1 = s["h1"]
            h2 = h2p.tile([P, KD, nb], BF16, tag="h2")
            for m in range(KD):
                ps = pmm.tile([P, nb], F32, tag="mm")
                for k in range(KF):
                    nc.tensor.matmul(ps[:], w2_sb[:, k, ts(m, P)], h1[:, k, :],
                                     start=(k == 0), stop=(k == KF - 1))
                if use_b2:
                    nc.scalar.activation(h2[:, m, :], ps[:], ACT.Identity,
                                         bias=b2_sb[:, m:m + 1])
                else:
                    nc.vector.tensor_copy(h2[:, m, :], ps[:])
            s["h2"] = h2

        def squares_out(i):
            s = st[i]
            s["sqo"] = sqo.tile([P, KD, nb], BF16, tag="sqo", name="sqot")
            for k in range(KD):
                nc.vector.tensor_mul(s["sqo"][:, k, :], s["h2"][:, k, :],
                                     s["h2"][:, k, :])

        def sums_out(i):
            s = st[i]
            ps_mu = pstat.tile([1, nb], F32, tag="somu")
            for k in range(KD):
                nc.tensor.matmul(ps_mu[:], cwm[:, 0, 0:1], s["h2"][:, k, :],
                                 start=(k == 0), stop=(k == KD - 1))
            ps_sq = pstat.tile([1, nb], F32, tag="sosq")
            for k in range(KD):
                nc.tensor.matmul(ps_sq[:], cwm[:, 0, 0:1], s["sqo"][:, k, :],
                                 start=(k == 0), stop=(k == KD - 1))
            s["ps_mu_o"], s["ps_sq_o"] = ps_mu, ps_sq

        def chain_out(i):
            s = st[i]
            ps_mu, ps_sq = s.pop("ps_mu_o"), s.pop("ps_sq_o")
            mu_sb = stp.tile([1, nb], F32, tag="omu_sb")
            nc.vector.tensor_copy(mu_sb[:], ps_mu[:])
            tmp = stp.tile([1, nb], F32, tag="otmp")
            nc.vector.tensor_mul(tmp[:], mu_sb[:], mu_sb[:])
            nc.vector.tensor_sub(tmp[:], ps_sq[:], tmp[:])
            nc.scalar.activation(tmp[:], tmp[:], ACT.Sqrt, bias=eps2[0:1, :])
            inv = stp.tile([1, nb], F32, tag="oinv")
            nc.vector.reciprocal_approx_fast(inv[:], tmp[:])
            s2t = stp.tile([1, 2, nb], BF16, tag="os2t")
            nc.vector.tensor_copy(s2t[:, 0, :], mu_sb[:])
            nc.vector.tensor_copy(s2t[:, 1, :], inv[:])
            s["obc"] = obcp.tile([P, 2, nb], BF16, tag="obc", name="obct")
            nc.gpsimd.partition_broadcast(s["obc"][:], s2t[:])

        def out_apply_store(i):
            s = st[i]
            sl = slice(i * nb, (i + 1) * nb)
            h2, obc = s["h2"], s["obc"]
            o = op_.tile([P, KD, nb], F32, tag="o")
            for k in range(KD):
                nc.vector.tensor_sub(o[:, k, :], h2[:, k, :], obc[:, 0, :])
                nc.vector.tensor_mul(o[:, k, :], o[:, k, :], obc[:, 1, :])
                if use_affine:
                    nc.vector.tensor_scalar(o[:, k, :], o[:, k, :],
                                            g_o_sb[:, k:k + 1],
                                            b_o_sb[:, k:k + 1],
                                            mybir.AluOpType.mult,
                                            mybir.AluOpType.add)
            nc.sync.dma_start(o_r[:, :, sl], o[:])

        # --- 3-deep software pipeline ---
        # iter i: attn/stats for tile i, FFN for tile i-2, out-LN for i-3
        # (the last tile's out-LN is inlined right after its FFN2).
        load(0)
        for i in range(nt + 2):
            if i + 1 < nt:
                load(i + 1)
            if i < nt:
                attn(i)
            if 0 <= i - 1 < nt:
                applies(i - 1)
            j = i - 3
            if 0 <= j < nt - 1:
                sums_out(j)
            if i < nt:
                squares(i)
            f = i - 2
            if 0 <= f < nt:
                ffn1(f)
            if i < nt:
                sums(i)
                chain(i)
            if 0 <= j < nt - 1:
                chain_out(j)
            if 0 <= f < nt:
                ffn2(f)
                squares_out(f)
                if f == nt - 1:
                    sums_out(f)
                    chain_out(f)
                    out_apply_store(f)
            if 0 <= j < nt - 1:
                out_apply_store(j)

    nc.compile()
    return nc


def kernel(**inputs) -> np.ndarray:
    global LAST_RESULTS
    f = lambda k: np.asarray(inputs[k], np.float32)

    drug = f("drug_emb")
    micro = f("micro_emb")
    b = drug.shape[0]
    bc = b // N_CORES
    assert b % (N_CORES * NB) == 0

    # ---- host-side weight folding ----
    wv_dm, bv_dm = f("dm_in_w")[2 * D:], f("dm_in_b")[2 * D:]
    wv_md, bv_md = f("md_in_w")[2 * D:], f("md_in_b")[2 * D:]
    a_dm = np.ascontiguousarray(wv_dm.T @ f("dm_out_w").T).astype(ml_dtypes.bfloat16)
    c_dm = bv_dm @ f("dm_out_w").T + f("dm_out_b")
    a_md = np.ascontiguousarray(wv_md.T @ f("md_out_w").T).astype(ml_dtypes.bfloat16)
    c_md = bv_md @ f("md_out_w").T + f("md_out_b")
    g_cat = np.concatenate([f("norm_d_g"), f("norm_m_g")])
    b_cat = np.concatenate([f("norm_d_b"), f("norm_m_b")])
    w1f = np.ascontiguousarray((f("ffn_w1") * g_cat[None, :]).T).astype(ml_dtypes.bfloat16)
    b1f = f("ffn_b1") + b_cat @ f("ffn_w1").T
    w2f = np.ascontiguousarray(f("ffn_w2").T).astype(ml_dtypes.bfloat16)
    b2 = f("ffn_b2")
    g_o, b_o = f("norm_out_g"), f("norm_out_b")

    flags = (bool(np.any(c_dm)), bool(np.any(c_md)), bool(np.any(b1f)),
             bool(np.any(b2)), bool(np.any(g_o != 1.0) or np.any(b_o)))

    key = (bc, NB, flags)
    if key not in _NC_CACHE:
        _NC_CACHE[key] = _build_nc(bc, NB, flags)
    nc = _NC_CACHE[key]

    in_maps = []
    for c in range(N_CORES):
        sl = slice(c * bc, (c + 1) * bc)
        m = {
            "xd": np.ascontiguousarray(drug[sl].T).astype(ml_dtypes.bfloat16),
            "xm": np.ascontiguousarray(micro[sl].T).astype(ml_dtypes.bfloat16),
            "a_dm": a_dm, "a_md": a_md, "w1": w1f, "w2": w2f,
        }
        if flags[0]:
            m["c_dm"] = c_dm
        if flags[1]:
            m["c_md"] = c_md
        if flags[2]:
            m["b1"] = b1f
        if flags[3]:
            m["b2"] = b2
        if flags[4]:
            m["g_o"] = g_o
            m["b_o"] = b_o
        in_maps.append(m)

    res = run_bass_kernel_spmd(nc, in_maps, list(range(N_CORES)))
    LAST_RESULTS = res

    out = np.empty((b, D), np.float32)
    for c in range(N_CORES):
        out[c * bc:(c + 1) * bc] = res.results[c]["o"].T
    return out


# revision 11
# speedup vs baseline: 1.0324x; 1.0324x over previous
"""CrossAttentionFusion forward on 8 Trainium2 NeuronCores (pure data parallel).

Math folded on host (seq-len-1 MHA == two chained linears):
  d_att = micro @ A_dm + c_dm,  A_dm = Wv_dm.T @ Wout_dm.T
  m_att = drug  @ A_md + c_md
  u = drug + d_att ; w = micro + m_att
  xu = (u - mu)/sd ; xw likewise        (LN affine folded into W1)
  h1 = gelu([xu, xw] @ W1f + b1f),  W1f = (ffn_w1 * g_cat).T
  h2 = h1 @ W2f + b2,               W2f = ffn_w2.T
  out = ((h2 - mu)/sd) * g_out + b_out

Device layout: activations feature-major [feat(partition), batch(free)];
batch sharded across 8 cores, tiles of NB=512 columns. LN stats via
M=2-packed ones-matmuls (sum and sumsq of both streams into two PSUM
banks); mu/rstd broadcast across partitions on the GpSimd engine
(partition_broadcast), keeping the tensor engine free of tiny matmuls.
The per-tile work is software-pipelined 3 deep so the tensor engine
never waits on the LN stats chain. All matmuls bf16 with fp32 PSUM.
"""

import sys

if "/opt/trn_rl_repo" not in sys.path:
    sys.path.insert(0, "/opt/trn_rl_repo")

from contextlib import ExitStack

import ml_dtypes
import numpy as np

import concourse.bass as bass  # noqa: F401  (registers mybir lowering hooks)
import concourse.tile as tile
from concourse import bacc, mybir
from concourse.bass import ts
from concourse.bass_utils import run_bass_kernel_spmd

F32 = mybir.dt.float32
BF16 = mybir.dt.bfloat16
ACT = mybir.ActivationFunctionType

P = 128
D = 384
KD = D // P          # 3
DH = 2 * D           # 768
KH = DH // P         # 6
DF = 4 * D           # 1536
KF = DF // P         # 12
EPS = 1e-5
N_CORES = 8
B_FULL = 65536
BC = B_FULL // N_CORES   # 8192 rows per core
NB = 512                 # batch columns per on-chip tile
SQD = float(1.0 / np.sqrt(D))

_NC_CACHE = {}
LAST_RESULTS = None      # BassKernelResults of the most recent kernel() call


def _build_nc(bc, nb, flags):
    use_c_dm, use_c_md, use_b1, use_b2, use_affine = flags
    nt = bc // nb
    assert nt >= 4
    nc = bacc.Bacc("TRN2", target_bir_lowering=False, debug=False,
                   num_devices=N_CORES)

    xd_d = nc.dram_tensor("xd", [D, bc], BF16, kind="ExternalInput")
    xm_d = nc.dram_tensor("xm", [D, bc], BF16, kind="ExternalInput")
    a_dm_d = nc.dram_tensor("a_dm", [D, D], BF16, kind="ExternalInput")
    a_md_d = nc.dram_tensor("a_md", [D, D], BF16, kind="ExternalInput")
    w1_d = nc.dram_tensor("w1", [DH, DF], BF16, kind="ExternalInput")
    w2_d = nc.dram_tensor("w2", [DF, D], BF16, kind="ExternalInput")
    c_dm_d = nc.dram_tensor("c_dm", [D], F32, kind="ExternalInput") if use_c_dm else None
    c_md_d = nc.dram_tensor("c_md", [D], F32, kind="ExternalInput") if use_c_md else None
    b1_d = nc.dram_tensor("b1", [DF], F32, kind="ExternalInput") if use_b1 else None
    b2_d = nc.dram_tensor("b2", [D], F32, kind="ExternalInput") if use_b2 else None
    g_o_d = nc.dram_tensor("g_o", [D], F32, kind="ExternalInput") if use_affine else None
    b_o_d = nc.dram_tensor("b_o", [D], F32, kind="ExternalInput") if use_affine else None
    o_d = nc.dram_tensor("o", [D, bc], F32, kind="ExternalOutput")

    xd_r = xd_d.ap().rearrange("(k p) n -> p k n", p=P)
    xm_r = xm_d.ap().rearrange("(k p) n -> p k n", p=P)
    o_r = o_d.ap().rearrange("(k p) n -> p k n", p=P)

    with tile.TileContext(nc) as tc, ExitStack() as ctx:
        wp = ctx.enter_context(tc.tile_pool(name="wts", bufs=1))
        xp = ctx.enter_context(tc.tile_pool(name="x", bufs=3))
        up = ctx.enter_context(tc.tile_pool(name="u", bufs=3))
        sqp = ctx.enter_context(tc.tile_pool(name="sq", bufs=2))
        xhp = ctx.enter_context(tc.tile_pool(name="xh", bufs=3))
        h1p = ctx.enter_context(tc.tile_pool(name="h1", bufs=2))
        h2p = ctx.enter_context(tc.tile_pool(name="h2", bufs=3))
        sqo = ctx.enter_context(tc.tile_pool(name="sqo", bufs=2))
        op_ = ctx.enter_context(tc.tile_pool(name="o", bufs=2))
        stp = ctx.enter_context(tc.tile_pool(name="st", bufs=1))
        bcp = ctx.enter_context(tc.tile_pool(name="bc", bufs=4))
        obcp = ctx.enter_context(tc.tile_pool(name="obc", bufs=2))
        pmm = ctx.enter_context(tc.tile_pool(name="pmm", bufs=4, space="PSUM"))
        pstat = ctx.enter_context(tc.tile_pool(name="pstat", bufs=1, space="PSUM"))

        a_dm_sb = wp.tile([P, KD, D], BF16)
        nc.gpsimd.dma_start(a_dm_sb[:], a_dm_d.ap().rearrange("(k p) m -> p k m", p=P))
        a_md_sb = wp.tile([P, KD, D], BF16)
        nc.gpsimd.dma_start(a_md_sb[:], a_md_d.ap().rearrange("(k p) m -> p k m", p=P))
        w1_sb = wp.tile([P, KH, DF], BF16)
        nc.gpsimd.dma_start(w1_sb[:], w1_d.ap().rearrange("(k p) m -> p k m", p=P))
        w2_sb = wp.tile([P, KF, D], BF16)
        nc.gpsimd.dma_start(w2_sb[:], w2_d.ap().rearrange("(k p) m -> p k m", p=P))

        # lhsT constants for the packed stat sums:
        #   cwm[:, s, :] : [P, 2] column s = 1/D      -> mean rows
        #   cws[:, s, :] : [P, 2] column s = 1.0      -> sumsq rows
        cwm = wp.tile([P, 2, 2], BF16)
        nc.vector.memset(cwm[:], 0.0)
        nc.vector.memset(cwm[:, 0, 0:1], 1.0 / D)
        nc.vector.memset(cwm[:, 1, 1:2], 1.0 / D)
        cws = wp.tile([P, 2, 2], BF16)
        nc.vector.memset(cws[:], 0.0)
        nc.vector.memset(cws[:, 0, 0:1], 1.0)
        nc.vector.memset(cws[:, 1, 1:2], 1.0)
        eps2 = wp.tile([2, 1], F32)
        nc.vector.memset(eps2[:], EPS)

        def vec_const(dram, nk, tag):
            t = wp.tile([P, nk], F32, tag=tag)
            nc.gpsimd.dma_start(t[:], dram.ap().rearrange("(k p) -> p k", p=P))
            return t

        c_dm_sb = vec_const(c_dm_d, KD, "c_dm") if use_c_dm else None
        c_md_sb = vec_const(c_md_d, KD, "c_md") if use_c_md else None
        b1_sb = vec_const(b1_d, KF, "b1") if use_b1 else None
        b2_sb = vec_const(b2_d, KD, "b2") if use_b2 else None
        g_o_sb = vec_const(g_o_d, KD, "g_o") if use_affine else None
        b_o_sb = vec_const(b_o_d, KD, "b_o") if use_affine else None

        st = [dict() for _ in range(nt)]

        def load(i):
            s = st[i]
            sl = slice(i * nb, (i + 1) * nb)
            s["xd"] = xp.tile([P, KD, nb], BF16, tag="xd", name="xdt")
            nc.sync.dma_start(s["xd"][:], xd_r[:, :, sl])
            s["xm"] = xp.tile([P, KD, nb], BF16, tag="xm", name="xmt")
            nc.sync.dma_start(s["xm"][:], xm_r[:, :, sl])

        def attn(i):
            s = st[i]

            def side(a_sb, rhs, res, c_sb, tag):
                v = up.tile([P, KD, nb], BF16, tag=tag)
                for m in range(KD):
                    ps = pmm.tile([P, nb], F32, tag="mm")
                    for k in range(KD):
                        nc.tensor.matmul(ps[:], a_sb[:, k, ts(m, P)], rhs[:, k, :],
                                         start=(k == 0), stop=(k == KD - 1))
                    nc.vector.tensor_add(v[:, m, :], ps[:], res[:, m, :])
                    if c_sb is not None:
                        nc.vector.tensor_scalar_add(v[:, m, :], v[:, m, :],
                                                    c_sb[:, m:m + 1])
                return v

            s["u"] = side(a_dm_sb, s["xm"], s["xd"], c_dm_sb, "u")
            s["w"] = side(a_md_sb, s["xd"], s["xm"], c_md_sb, "w")

        def squares(i):
            s = st[i]
            s["squ"] = sqp.tile([P, KD, nb], BF16, tag="squ", name="squ")
            s["sqw"] = sqp.tile([P, KD, nb], BF16, tag="sqw", name="sqw")
            for k in range(KD):
                nc.scalar.activation(s["squ"][:, k, :], s["u"][:, k, :],
                                     ACT.Square, scale=SQD)
                nc.scalar.activation(s["sqw"][:, k, :], s["w"][:, k, :],
                                     ACT.Square, scale=SQD)

        def sums(i):
            """ps_mu rows [mu_u, mu_w]; ps_sq rows [E[u^2], E[w^2]]."""
            s = st[i]
            ps_mu = pstat.tile([2, nb], F32, tag="smu")
            n = 0
            for sdx, x in ((0, s["u"]), (1, s["w"])):
                for k in range(KD):
                    nc.tensor.matmul(ps_mu[:], cwm[:, sdx, :], x[:, k, :],
                                     start=(n == 0), stop=(n == 2 * KD - 1))
                    n += 1
            ps_sq = pstat.tile([2, nb], F32, tag="ssq")
            n = 0
            for sdx, x in ((0, s["squ"]), (1, s["sqw"])):
                for k in range(KD):
                    nc.tensor.matmul(ps_sq[:], cws[:, sdx, :], x[:, k, :],
                                     start=(n == 0), stop=(n == 2 * KD - 1))
                    n += 1
            s["ps_mu"], s["ps_sq"] = ps_mu, ps_sq

        def chain(i):
            """stats chain on partitions 0..1; broadcast via GpSimd."""
            s = st[i]
            ps_mu, ps_sq = s.pop("ps_mu"), s.pop("ps_sq")
            mu_sb = stp.tile([2, nb], F32, tag="mu_sb")
            nc.vector.tensor_copy(mu_sb[:], ps_mu[:])
            tmp = stp.tile([2, nb], F32, tag="tmp")
            nc.vector.tensor_mul(tmp[:], mu_sb[:], mu_sb[:])
            nc.vector.tensor_sub(tmp[:], ps_sq[:], tmp[:])
            nc.scalar.activation(tmp[:], tmp[:], ACT.Sqrt, bias=eps2[:])
            inv = stp.tile([2, nb], F32, tag="inv")
            nc.vector.reciprocal_approx_fast(inv[:], tmp[:])
            s2t = stp.tile([2, 2, nb], BF16, tag="s2t")
            nc.vector.tensor_copy(s2t[:, 0, :], mu_sb[:])
            nc.vector.tensor_copy(s2t[:, 1, :], inv[:])
            # GpSimd ops must start at partition 0; DMA row 1 down first.
            s2tw = stp.tile([1, 2, nb], BF16, tag="s2tw")
            nc.sync.dma_start(s2tw[:], s2t[1:2, :, :])
            s["bcu"] = bcp.tile([P, 2, nb], BF16, tag="bc", name="bcu")
            nc.gpsimd.partition_broadcast(s["bcu"][:], s2t[0:1, :, :])
            s["bcw"] = bcp.tile([P, 2, nb], BF16, tag="bc", name="bcw")
            nc.gpsimd.partition_broadcast(s["bcw"][:], s2tw[:])

        def applies(i):
            s = st[i]
            s["xhu"] = xhp.tile([P, KD, nb], BF16, tag="xhu", name="xhu")
            s["xhw"] = xhp.tile([P, KD, nb], BF16, tag="xhw", name="xhw")
            for x, bc, xh in ((s["u"], s["bcu"], s["xhu"]),
                              (s["w"], s["bcw"], s["xhw"])):
                for k in range(KD):
                    nc.vector.tensor_sub(xh[:, k, :], x[:, k, :], bc[:, 0, :])
                    nc.vector.tensor_mul(xh[:, k, :], xh[:, k, :], bc[:, 1, :])

        def ffn1(i):
            s = st[i]
            xhu, xhw = s["xhu"], s["xhw"]
            h1 = h1p.tile([P, KF, nb], BF16, tag="h1")
            for m in range(KF):
                ps = pmm.tile([P, nb], F32, tag="mm")
                for k in range(KH):
                    rhs = xhu[:, k, :] if k < KD else xhw[:, k - KD, :]
                    nc.tensor.matmul(ps[:], w1_sb[:, k, ts(m, P)], rhs,
                                     start=(k == 0), stop=(k == KH - 1))
                if use_b1:
                    nc.scalar.activation(h1[:, m, :], ps[:], ACT.Gelu,
                                         bias=b1_sb[:, m:m + 1])
                else:
                    nc.scalar.activation(h1[:, m, :], ps[:], ACT.Gelu)
            s["h1"] = h1

        def ffn2(i):
            s = st[i]
            h1 = s["h1"]
            h2 = h2p.tile([P, KD, nb], BF16, tag="h2")
            for m in range(KD):
                ps = pmm.tile([P, nb], F32, tag="mm")
                for k in range(KF):
                    nc.tensor.matmul(ps[:], w2_sb[:, k, ts(m, P)], h1[:, k, :],
                                     start=(k == 0), stop=(k == KF - 1))
                if use_b2:
                    nc.scalar.activation(h2[:, m, :], ps[:], ACT.Identity,
                                         bias=b2_sb[:, m:m + 1])
                else:
                    nc.scalar.activation(h2[:, m, :], ps[:], ACT.Copy)
            s["h2"] = h2

        def squares_out(i):
            s = st[i]
            s["sqo"] = sqo.tile([P, KD, nb], BF16, tag="sqo", name="sqot")
            for k in range(KD):
                nc.scalar.activation(s["sqo"][:, k, :], s["h2"][:, k, :],
                                     ACT.Square, scale=SQD)

        def sums_out(i):
            s = st[i]
            ps_mu = pstat.tile([1, nb], F32, tag="somu")
            for k in range(KD):
                nc.tensor.matmul(ps_mu[:], cwm[:, 0, 0:1], s["h2"][:, k, :],
                                 start=(k == 0), stop=(k == KD - 1))
            ps_sq = pstat.tile([1, nb], F32, tag="sosq")
            for k in range(KD):
                nc.tensor.matmul(ps_sq[:], cws[:, 0, 0:1], s["sqo"][:, k, :],
                                 start=(k == 0), stop=(k == KD - 1))
            s["ps_mu_o"], s["ps_sq_o"] = ps_mu, ps_sq

        def chain_out(i):
            s = st[i]
            ps_mu, ps_sq = s.pop("ps_mu_o"), s.pop("ps_sq_o")
            mu_sb = stp.tile([1, nb], F32, tag="omu_sb")
            nc.vector.tensor_copy(mu_sb[:], ps_mu[:])
            tmp = stp.tile([1, nb], F32, tag="otmp")
            nc.vector.tensor_mul(tmp[:], mu_sb[:], mu_sb[:])
            nc.vector.tensor_sub(tmp[:], ps_sq[:], tmp[:])
            nc.scalar.activation(tmp[:], tmp[:], ACT.Sqrt, bias=eps2[0:1, :])
            inv = stp.tile([1, nb], F32, tag="oinv")
            nc.vector.reciprocal_approx_fast(inv[:], tmp[:])
            s2t = stp.tile([1, 2, nb], BF16, tag="os2t")
            nc.vector.tensor_copy(s2t[:, 0, :], mu_sb[:])
            nc.vector.tensor_copy(s2t[:, 1, :], inv[:])
            s["obc"] = obcp.tile([P, 2, nb], BF16, tag="obc", name="obct")
            nc.gpsimd.partition_broadcast(s["obc"][:], s2t[:])

        def out_apply_store(i):
            s = st[i]
            sl = slice(i * nb, (i + 1) * nb)
            h2, obc = s["h2"], s["obc"]
            o = op_.tile([P, KD, nb], F32, tag="o")
            for k in range(KD):
                nc.vector.tensor_sub(o[:, k, :], h2[:, k, :], obc[:, 0, :])
                nc.vector.tensor_mul(o[:, k, :], o[:, k, :], obc[:, 1, :])
                if use_affine:
                    nc.vector.tensor_scalar(o[:, k, :], o[:, k, :],
                                            g_o_sb[:, k:k + 1],
                                            b_o_sb[:, k:k + 1],
                                            mybir.AluOpType.mult,
                                            mybir.AluOpType.add)
            nc.sync.dma_start(o_r[:, :, sl], o[:])

        # --- 3-deep software pipeline ---
        # iter i: attn/stats for tile i, FFN for tile i-2, out-LN for i-3
        # (the last tile's out-LN is inlined right after its FFN2).
        load(0)
        for i in range(nt + 2):
            if i + 1 < nt:
                load(i + 1)
            if i < nt:
                attn(i)
            if 0 <= i - 1 < nt:
                applies(i - 1)
            j = i - 3
            if 0 <= j < nt - 1:
                sums_out(j)
            if i < nt:
                squares(i)
            f = i - 2
            if 0 <= f < nt:
                ffn1(f)
            if i < nt:
                sums(i)
                chain(i)
            if 0 <= j < nt - 1:
                chain_out(j)
            if 0 <= f < nt:
                ffn2(f)
                squares_out(f)
                if f == nt - 1:
                    sums_out(f)
                    chain_out(f)
                    out_apply_store(f)
            if 0 <= j < nt - 1:
                out_apply_store(j)

    nc.compile()
    return nc


def kernel(**inputs) -> np.ndarray:
    global LAST_RESULTS
    f = lambda k: np.asarray(inputs[k], np.float32)

    drug = f("drug_emb")
    micro = f("micro_emb")
    b = drug.shape[0]
    bc = b // N_CORES
    assert b % (N_CORES * NB) == 0

    # ---- host-side weight folding ----
    wv_dm, bv_dm = f("dm_in_w")[2 * D:], f("dm_in_b")[2 * D:]
    wv_md, bv_md = f("md_in_w")[2 * D:], f("md_in_b")[2 * D:]
    a_dm = np.ascontiguousarray(wv_dm.T @ f("dm_out_w").T).astype(ml_dtypes.bfloat16)
    c_dm = bv_dm @ f("dm_out_w").T + f("dm_out_b")
    a_md = np.ascontiguousarray(wv_md.T @ f("md_out_w").T).astype(ml_dtypes.bfloat16)
    c_md = bv_md @ f("md_out_w").T + f("md_out_b")
    g_cat = np.concatenate([f("norm_d_g"), f("norm_m_g")])
    b_cat = np.concatenate([f("norm_d_b"), f("norm_m_b")])
    w1f = np.ascontiguousarray((f("ffn_w1") * g_cat[None, :]).T).astype(ml_dtypes.bfloat16)
    b1f = f("ffn_b1") + b_cat @ f("ffn_w1").T
    w2f = np.ascontiguousarray(f("ffn_w2").T).astype(ml_dtypes.bfloat16)
    b2 = f("ffn_b2")
    g_o, b_o = f("norm_out_g"), f("norm_out_b")

    flags = (bool(np.any(c_dm)), bool(np.any(c_md)), bool(np.any(b1f)),
             bool(np.any(b2)), bool(np.any(g_o != 1.0) or np.any(b_o)))

    key = (bc, NB, flags)
    if key not in _NC_CACHE:
        _NC_CACHE[key] = _build_nc(bc, NB, flags)
    nc = _NC_CACHE[key]

    in_maps = []
    for c in range(N_CORES):
        sl = slice(c * bc, (c + 1) * bc)
        m = {
            "xd": np.ascontiguousarray(drug[sl].T).astype(ml_dtypes.bfloat16),
            "xm": np.ascontiguousarray(micro[sl].T).astype(ml_dtypes.bfloat16),
            "a_dm": a_dm, "a_md": a_md, "w1": w1f, "w2": w2f,
        }
        if flags[0]:
            m["c_dm"] = c_dm
        if flags[1]:
            m["c_md"] = c_md
        if flags[2]:
            m["b1"] = b1f
        if flags[3]:
            m["b2"] = b2
        if flags[4]:
            m["g_o"] = g_o
            m["b_o"] = b_o
        in_maps.append(m)

    res = run_bass_kernel_spmd(nc, in_maps, list(range(N_CORES)))
    LAST_RESULTS = res

    out = np.empty((b, D), np.float32)
    for c in range(N_CORES):
        out[c * bc:(c + 1) * bc] = res.results[c]["o"].T
    return out


# revision 12
# speedup vs baseline: 1.0392x; 1.0066x over previous
"""CrossAttentionFusion forward on 8 Trainium2 NeuronCores (pure data parallel).

Math folded on host (seq-len-1 MHA == two chained linears):
  d_att = micro @ A_dm + c_dm,  A_dm = Wv_dm.T @ Wout_dm.T
  m_att = drug  @ A_md + c_md
  u = drug + d_att ; w = micro + m_att
  xu = (u - mu)/sd ; xw likewise        (LN affine folded into W1)
  h1 = gelu([xu, xw] @ W1f + b1f),  W1f = (ffn_w1 * g_cat).T
  h2 = h1 @ W2f + b2,               W2f = ffn_w2.T
  out = ((h2 - mu)/sd) * g_out + b_out

Device layout: activations feature-major [feat(partition), batch(free)];
batch sharded across 8 cores, tiles of NB=512 columns. LN stats via
M=2-packed ones-matmuls (sum and sumsq of both streams into two PSUM
banks); mu/rstd broadcast across partitions on the GpSimd engine
(partition_broadcast), keeping the tensor engine free of tiny matmuls.
The per-tile work is software-pipelined 3 deep so the tensor engine
never waits on the LN stats chain. All matmuls bf16 with fp32 PSUM.
"""

import sys

if "/opt/trn_rl_repo" not in sys.path:
    sys.path.insert(0, "/opt/trn_rl_repo")

from contextlib import ExitStack

import ml_dtypes
import numpy as np

import concourse.bass as bass  # noqa: F401  (registers mybir lowering hooks)
import concourse.tile as tile
from concourse import bacc, mybir
from concourse.bass import ts
from concourse.bass_utils import run_bass_kernel_spmd

F32 = mybir.dt.float32
BF16 = mybir.dt.bfloat16
ACT = mybir.ActivationFunctionType

P = 128
D = 384
KD = D // P          # 3
DH = 2 * D           # 768
KH = DH // P         # 6
DF = 4 * D           # 1536
KF = DF // P         # 12
EPS = 1e-5
N_CORES = 8
B_FULL = 65536
BC = B_FULL // N_CORES   # 8192 rows per core
NB = 512                 # batch columns per on-chip tile
SQD = float(1.0 / np.sqrt(D))

_NC_CACHE = {}
LAST_RESULTS = None      # BassKernelResults of the most recent kernel() call


def _build_nc(bc, nb, flags):
    use_c_dm, use_c_md, use_b1, use_b2, use_affine = flags
    nt = bc // nb
    assert nt >= 4
    nc = bacc.Bacc("TRN2", target_bir_lowering=False, debug=False,
                   num_devices=N_CORES)

    xd_d = nc.dram_tensor("xd", [D, bc], BF16, kind="ExternalInput")
    xm_d = nc.dram_tensor("xm", [D, bc], BF16, kind="ExternalInput")
    a_dm_d = nc.dram_tensor("a_dm", [D, D], BF16, kind="ExternalInput")
    a_md_d = nc.dram_tensor("a_md", [D, D], BF16, kind="ExternalInput")
    w1_d = nc.dram_tensor("w1", [DH, DF], BF16, kind="ExternalInput")
    w2_d = nc.dram_tensor("w2", [DF, D], BF16, kind="ExternalInput")
    c_dm_d = nc.dram_tensor("c_dm", [D], F32, kind="ExternalInput") if use_c_dm else None
    c_md_d = nc.dram_tensor("c_md", [D], F32, kind="ExternalInput") if use_c_md else None
    b1_d = nc.dram_tensor("b1", [DF], F32, kind="ExternalInput") if use_b1 else None
    b2_d = nc.dram_tensor("b2", [D], F32, kind="ExternalInput") if use_b2 else None
    g_o_d = nc.dram_tensor("g_o", [D], F32, kind="ExternalInput") if use_affine else None
    b_o_d = nc.dram_tensor("b_o", [D], F32, kind="ExternalInput") if use_affine else None
    o_d = nc.dram_tensor("o", [D, bc], F32, kind="ExternalOutput")

    xd_r = xd_d.ap().rearrange("(k p) n -> p k n", p=P)
    xm_r = xm_d.ap().rearrange("(k p) n -> p k n", p=P)
    o_r = o_d.ap().rearrange("(k p) n -> p k n", p=P)

    with tile.TileContext(nc) as tc, ExitStack() as ctx:
        wp = ctx.enter_context(tc.tile_pool(name="wts", bufs=1))
        xp = ctx.enter_context(tc.tile_pool(name="x", bufs=3))
        up = ctx.enter_context(tc.tile_pool(name="u", bufs=5))
        sqp = ctx.enter_context(tc.tile_pool(name="sq", bufs=3))
        xhp = ctx.enter_context(tc.tile_pool(name="xh", bufs=3))
        h1p = ctx.enter_context(tc.tile_pool(name="h1", bufs=2))
        h2p = ctx.enter_context(tc.tile_pool(name="h2", bufs=3))
        sqo = ctx.enter_context(tc.tile_pool(name="sqo", bufs=2))
        op_ = ctx.enter_context(tc.tile_pool(name="o", bufs=2))
        stp = ctx.enter_context(tc.tile_pool(name="st", bufs=1))
        bcp = ctx.enter_context(tc.tile_pool(name="bc", bufs=4))
        obcp = ctx.enter_context(tc.tile_pool(name="obc", bufs=2))
        pmm = ctx.enter_context(tc.tile_pool(name="pmm", bufs=4, space="PSUM"))
        pstat = ctx.enter_context(tc.tile_pool(name="pstat", bufs=1, space="PSUM"))

        a_dm_sb = wp.tile([P, KD, D], BF16)
        nc.gpsimd.dma_start(a_dm_sb[:], a_dm_d.ap().rearrange("(k p) m -> p k m", p=P))
        a_md_sb = wp.tile([P, KD, D], BF16)
        nc.gpsimd.dma_start(a_md_sb[:], a_md_d.ap().rearrange("(k p) m -> p k m", p=P))
        w1_sb = wp.tile([P, KH, DF], BF16)
        nc.gpsimd.dma_start(w1_sb[:], w1_d.ap().rearrange("(k p) m -> p k m", p=P))
        w2_sb = wp.tile([P, KF, D], BF16)
        nc.gpsimd.dma_start(w2_sb[:], w2_d.ap().rearrange("(k p) m -> p k m", p=P))

        # lhsT constants for the packed stat sums:
        #   cwm[:, s, :] : [P, 2] column s = 1/D      -> mean rows
        #   cws[:, s, :] : [P, 2] column s = 1.0      -> sumsq rows
        cwm = wp.tile([P, 2, 2], BF16)
        nc.vector.memset(cwm[:], 0.0)
        nc.vector.memset(cwm[:, 0, 0:1], 1.0 / D)
        nc.vector.memset(cwm[:, 1, 1:2], 1.0 / D)
        cws = wp.tile([P, 2, 2], BF16)
        nc.vector.memset(cws[:], 0.0)
        nc.vector.memset(cws[:, 0, 0:1], 1.0)
        nc.vector.memset(cws[:, 1, 1:2], 1.0)
        eps2 = wp.tile([2, 1], F32)
        nc.vector.memset(eps2[:], EPS)

        def vec_const(dram, nk, tag):
            t = wp.tile([P, nk], F32, tag=tag)
            nc.gpsimd.dma_start(t[:], dram.ap().rearrange("(k p) -> p k", p=P))
            return t

        c_dm_sb = vec_const(c_dm_d, KD, "c_dm") if use_c_dm else None
        c_md_sb = vec_const(c_md_d, KD, "c_md") if use_c_md else None
        b1_sb = vec_const(b1_d, KF, "b1") if use_b1 else None
        b2_sb = vec_const(b2_d, KD, "b2") if use_b2 else None
        g_o_sb = vec_const(g_o_d, KD, "g_o") if use_affine else None
        b_o_sb = vec_const(b_o_d, KD, "b_o") if use_affine else None

        st = [dict() for _ in range(nt)]

        def load(i):
            s = st[i]
            sl = slice(i * nb, (i + 1) * nb)
            s["xd"] = xp.tile([P, KD, nb], BF16, tag="xd", name="xdt")
            nc.sync.dma_start(s["xd"][:], xd_r[:, :, sl])
            s["xm"] = xp.tile([P, KD, nb], BF16, tag="xm", name="xmt")
            nc.sync.dma_start(s["xm"][:], xm_r[:, :, sl])

        def attn(i):
            s = st[i]

            def side(a_sb, rhs, res, c_sb, tag):
                v = up.tile([P, KD, nb], BF16, tag=tag)
                for m in range(KD):
                    ps = pmm.tile([P, nb], F32, tag="mm")
                    for k in range(KD):
                        nc.tensor.matmul(ps[:], a_sb[:, k, ts(m, P)], rhs[:, k, :],
                                         start=(k == 0), stop=(k == KD - 1))
                    nc.vector.tensor_add(v[:, m, :], ps[:], res[:, m, :])
                    if c_sb is not None:
                        nc.vector.tensor_scalar_add(v[:, m, :], v[:, m, :],
                                                    c_sb[:, m:m + 1])
                return v

            s["u"] = side(a_dm_sb, s["xm"], s["xd"], c_dm_sb, "u")
            s["w"] = side(a_md_sb, s["xd"], s["xm"], c_md_sb, "w")

        def squares(i):
            s = st[i]
            s["squ"] = sqp.tile([P, KD, nb], BF16, tag="squ", name="squ")
            s["sqw"] = sqp.tile([P, KD, nb], BF16, tag="sqw", name="sqw")
            for k in range(KD):
                nc.scalar.activation(s["squ"][:, k, :], s["u"][:, k, :],
                                     ACT.Square, scale=SQD)
                nc.scalar.activation(s["sqw"][:, k, :], s["w"][:, k, :],
                                     ACT.Square, scale=SQD)

        def sums(i):
            """ps_mu rows [mu_u, mu_w]; ps_sq rows [E[u^2], E[w^2]]."""
            s = st[i]
            ps_mu = pstat.tile([2, nb], F32, tag="smu")
            n = 0
            for sdx, x in ((0, s["u"]), (1, s["w"])):
                for k in range(KD):
                    nc.tensor.matmul(ps_mu[:], cwm[:, sdx, :], x[:, k, :],
                                     start=(n == 0), stop=(n == 2 * KD - 1))
                    n += 1
            ps_sq = pstat.tile([2, nb], F32, tag="ssq")
            n = 0
            for sdx, x in ((0, s["squ"]), (1, s["sqw"])):
                for k in range(KD):
                    nc.tensor.matmul(ps_sq[:], cws[:, sdx, :], x[:, k, :],
                                     start=(n == 0), stop=(n == 2 * KD - 1))
                    n += 1
            s["ps_mu"], s["ps_sq"] = ps_mu, ps_sq

        def chain(i):
            """stats chain on partitions 0..1; broadcast via GpSimd."""
            s = st[i]
            ps_mu, ps_sq = s.pop("ps_mu"), s.pop("ps_sq")
            mu_sb = stp.tile([2, nb], F32, tag="mu_sb")
            nc.vector.tensor_copy(mu_sb[:], ps_mu[:])
            tmp = stp.tile([2, nb], F32, tag="tmp")
            nc.vector.tensor_mul(tmp[:], mu_sb[:], mu_sb[:])
            nc.vector.tensor_sub(tmp[:], ps_sq[:], tmp[:])
            nc.scalar.activation(tmp[:], tmp[:], ACT.Sqrt, bias=eps2[:])
            inv = stp.tile([2, nb], F32, tag="inv")
            nc.vector.reciprocal_approx_fast(inv[:], tmp[:])
            s2t = stp.tile([2, 2, nb], BF16, tag="s2t")
            nc.vector.tensor_copy(s2t[:, 0, :], mu_sb[:])
            nc.vector.tensor_copy(s2t[:, 1, :], inv[:])
            # GpSimd ops must start at partition 0; DMA row 1 down first.
            s2tw = stp.tile([1, 2, nb], BF16, tag="s2tw")
            nc.sync.dma_start(s2tw[:], s2t[1:2, :, :])
            s["bcu"] = bcp.tile([P, 2, nb], BF16, tag="bc", name="bcu")
            nc.gpsimd.partition_broadcast(s["bcu"][:], s2t[0:1, :, :])
            s["bcw"] = bcp.tile([P, 2, nb], BF16, tag="bc", name="bcw")
            nc.gpsimd.partition_broadcast(s["bcw"][:], s2tw[:])

        def applies(i):
            s = st[i]
            s["xhu"] = xhp.tile([P, KD, nb], BF16, tag="xhu", name="xhu")
            s["xhw"] = xhp.tile([P, KD, nb], BF16, tag="xhw", name="xhw")
            for x, bc, xh in ((s["u"], s["bcu"], s["xhu"]),
                              (s["w"], s["bcw"], s["xhw"])):
                for k in range(KD):
                    nc.vector.tensor_sub(xh[:, k, :], x[:, k, :], bc[:, 0, :])
                    nc.vector.tensor_mul(xh[:, k, :], xh[:, k, :], bc[:, 1, :])

        def ffn1(i):
            s = st[i]
            xhu, xhw = s["xhu"], s["xhw"]
            h1 = h1p.tile([P, KF, nb], BF16, tag="h1")
            for m in range(KF):
                ps = pmm.tile([P, nb], F32, tag="mm")
                for k in range(KH):
                    rhs = xhu[:, k, :] if k < KD else xhw[:, k - KD, :]
                    nc.tensor.matmul(ps[:], w1_sb[:, k, ts(m, P)], rhs,
                                     start=(k == 0), stop=(k == KH - 1))
                if use_b1:
                    nc.scalar.activation(h1[:, m, :], ps[:], ACT.Gelu,
                                         bias=b1_sb[:, m:m + 1])
                else:
                    nc.scalar.activation(h1[:, m, :], ps[:], ACT.Gelu)
            s["h1"] = h1

        def ffn2(i):
            s = st[i]
            h1 = s["h1"]
            h2 = h2p.tile([P, KD, nb], BF16, tag="h2")
            for m in range(KD):
                ps = pmm.tile([P, nb], F32, tag="mm")
                for k in range(KF):
                    nc.tensor.matmul(ps[:], w2_sb[:, k, ts(m, P)], h1[:, k, :],
                                     start=(k == 0), stop=(k == KF - 1))
                if use_b2:
                    nc.scalar.activation(h2[:, m, :], ps[:], ACT.Identity,
                                         bias=b2_sb[:, m:m + 1])
                else:
                    nc.scalar.activation(h2[:, m, :], ps[:], ACT.Copy)
            s["h2"] = h2

        def squares_out(i):
            s = st[i]
            s["sqo"] = sqo.tile([P, KD, nb], BF16, tag="sqo", name="sqot")
            for k in range(KD):
                nc.scalar.activation(s["sqo"][:, k, :], s["h2"][:, k, :],
                                     ACT.Square, scale=SQD)

        def sums_out(i):
            s = st[i]
            ps_mu = pstat.tile([1, nb], F32, tag="somu")
            for k in range(KD):
                nc.tensor.matmul(ps_mu[:], cwm[:, 0, 0:1], s["h2"][:, k, :],
                                 start=(k == 0), stop=(k == KD - 1))
            ps_sq = pstat.tile([1, nb], F32, tag="sosq")
            for k in range(KD):
                nc.tensor.matmul(ps_sq[:], cws[:, 0, 0:1], s["sqo"][:, k, :],
                                 start=(k == 0), stop=(k == KD - 1))
            s["ps_mu_o"], s["ps_sq_o"] = ps_mu, ps_sq

        def chain_out(i):
            s = st[i]
            ps_mu, ps_sq = s.pop("ps_mu_o"), s.pop("ps_sq_o")
            mu_sb = stp.tile([1, nb], F32, tag="omu_sb")
            nc.vector.tensor_copy(mu_sb[:], ps_mu[:])
            tmp = stp.tile([1, nb], F32, tag="otmp")
            nc.vector.tensor_mul(tmp[:], mu_sb[:], mu_sb[:])
            nc.vector.tensor_sub(tmp[:], ps_sq[:], tmp[:])
            nc.scalar.activation(tmp[:], tmp[:], ACT.Sqrt, bias=eps2[0:1, :])
            inv = stp.tile([1, nb], F32, tag="oinv")
            nc.vector.reciprocal_approx_fast(inv[:], tmp[:])
            s2t = stp.tile([1, 2, nb], BF16, tag="os2t")
            nc.vector.tensor_copy(s2t[:, 0, :], mu_sb[:])
            nc.vector.tensor_copy(s2t[:, 1, :], inv[:])
            s["obc"] = obcp.tile([P, 2, nb], BF16, tag="obc", name="obct")
            nc.gpsimd.partition_broadcast(s["obc"][:], s2t[:])

        def out_apply_store(i):
            s = st[i]
            sl = slice(i * nb, (i + 1) * nb)
            h2, obc = s["h2"], s["obc"]
            o = op_.tile([P, KD, nb], F32, tag="o")
            for k in range(KD):
                nc.vector.tensor_sub(o[:, k, :], h2[:, k, :], obc[:, 0, :])
                nc.vector.tensor_mul(o[:, k, :], o[:, k, :], obc[:, 1, :])
                if use_affine:
                    nc.vector.tensor_scalar(o[:, k, :], o[:, k, :],
                                            g_o_sb[:, k:k + 1],
                                            b_o_sb[:, k:k + 1],
                                            mybir.AluOpType.mult,
                                            mybir.AluOpType.add)
                nc.sync.dma_start(o_r[:, k, sl], o[:, k, :])

        # --- software pipeline, attention 2 iters ahead ---
        # iter i: attn for tile i+2, stats for tile i, FFN for tile i-2,
        # out-LN for tile i-3.  Deep attn keeps the PE dense (and the HAM
        # clock warm) while tile 0's serial LN chain resolves.
        for t in range(min(3, nt)):
            load(t)
        for t in range(min(2, nt)):
            attn(t)
            squares(t)
        for i in range(nt + 3):
            if i + 3 < nt:
                load(i + 3)
            if i + 2 < nt:
                attn(i + 2)
                squares(i + 2)
            j = i - 3
            if j >= 0:
                sums_out(j)
                chain_out(j)
            f = i - 2
            if 0 <= f < nt:
                ffn1(f)
            if i < nt:
                sums(i)
                chain(i)
            if 0 <= f < nt:
                ffn2(f)
                squares_out(f)
            if 0 <= i - 1 < nt:
                applies(i - 1)
            if j >= 0:
                out_apply_store(j)

    nc.compile()
    return nc


def kernel(**inputs) -> np.ndarray:
    global LAST_RESULTS
    f = lambda k: np.asarray(inputs[k], np.float32)

    drug = f("drug_emb")
    micro = f("micro_emb")
    b = drug.shape[0]
    bc = b // N_CORES
    assert b % (N_CORES * NB) == 0

    # ---- host-side weight folding ----
    wv_dm, bv_dm = f("dm_in_w")[2 * D:], f("dm_in_b")[2 * D:]
    wv_md, bv_md = f("md_in_w")[2 * D:], f("md_in_b")[2 * D:]
    a_dm = np.ascontiguousarray(wv_dm.T @ f("dm_out_w").T).astype(ml_dtypes.bfloat16)
    c_dm = bv_dm @ f("dm_out_w").T + f("dm_out_b")
    a_md = np.ascontiguousarray(wv_md.T @ f("md_out_w").T).astype(ml_dtypes.bfloat16)
    c_md = bv_md @ f("md_out_w").T + f("md_out_b")
    g_cat = np.concatenate([f("norm_d_g"), f("norm_m_g")])
    b_cat = np.concatenate([f("norm_d_b"), f("norm_m_b")])
    w1f = np.ascontiguousarray((f("ffn_w1") * g_cat[None, :]).T).astype(ml_dtypes.bfloat16)
    b1f = f("ffn_b1") + b_cat @ f("ffn_w1").T
    w2f = np.ascontiguousarray(f("ffn_w2").T).astype(ml_dtypes.bfloat16)
    b2 = f("ffn_b2")
    g_o, b_o = f("norm_out_g"), f("norm_out_b")

    flags = (bool(np.any(c_dm)), bool(np.any(c_md)), bool(np.any(b1f)),
             bool(np.any(b2)), bool(np.any(g_o != 1.0) or np.any(b_o)))

    key = (bc, NB, flags)
    if key not in _NC_CACHE:
        _NC_CACHE[key] = _build_nc(bc, NB, flags)
    nc = _NC_CACHE[key]

    in_maps = []
    for c in range(N_CORES):
        sl = slice(c * bc, (c + 1) * bc)
        m = {
            "xd": np.ascontiguousarray(drug[sl].T).astype(ml_dtypes.bfloat16),
            "xm": np.ascontiguousarray(micro[sl].T).astype(ml_dtypes.bfloat16),
            "a_dm": a_dm, "a_md": a_md, "w1": w1f, "w2": w2f,
        }
        if flags[0]:
            m["c_dm"] = c_dm
        if flags[1]:
            m["c_md"] = c_md
        if flags[2]:
            m["b1"] = b1f
        if flags[3]:
            m["b2"] = b2
        if flags[4]:
            m["g_o"] = g_o
            m["b_o"] = b_o
        in_maps.append(m)

    res = run_bass_kernel_spmd(nc, in_maps, list(range(N_CORES)))
    LAST_RESULTS = res

    out = np.empty((b, D), np.float32)
    for c in range(N_CORES):
        out[c * bc:(c + 1) * bc] = res.results[c]["o"].T
    return out


# revision 13
# speedup vs baseline: 1.0397x; 1.0004x over previous
"""CrossAttentionFusion forward on 8 Trainium2 NeuronCores (pure data parallel).

Math folded on host (seq-len-1 MHA == two chained linears):
  d_att = micro @ A_dm + c_dm,  A_dm = Wv_dm.T @ Wout_dm.T
  m_att = drug  @ A_md + c_md
  u = drug + d_att ; w = micro + m_att
  xu = (u - mu)/sd ; xw likewise        (LN affine folded into W1)
  h1 = gelu([xu, xw] @ W1f + b1f),  W1f = (ffn_w1 * g_cat).T
  h2 = h1 @ W2f + b2,               W2f = ffn_w2.T
  out = ((h2 - mu)/sd) * g_out + b_out

Device layout: activations feature-major [feat(partition), batch(free)];
batch sharded across 8 cores, tiles of NB=512 columns. LN stats via
M=2-packed ones-matmuls (sum and sumsq of both streams into two PSUM
banks); mu/rstd broadcast across partitions on the GpSimd engine
(partition_broadcast), keeping the tensor engine free of tiny matmuls.
The per-tile work is software-pipelined 3 deep so the tensor engine
never waits on the LN stats chain. All matmuls bf16 with fp32 PSUM.
"""

import sys

if "/opt/trn_rl_repo" not in sys.path:
    sys.path.insert(0, "/opt/trn_rl_repo")

from contextlib import ExitStack

import ml_dtypes
import numpy as np

import concourse.bass as bass  # noqa: F401  (registers mybir lowering hooks)
import concourse.tile as tile
from concourse import bacc, mybir
from concourse.bass import ts
from concourse.bass_utils import run_bass_kernel_spmd

F32 = mybir.dt.float32
BF16 = mybir.dt.bfloat16
ACT = mybir.ActivationFunctionType

P = 128
D = 384
KD = D // P          # 3
DH = 2 * D           # 768
KH = DH // P         # 6
DF = 4 * D           # 1536
KF = DF // P         # 12
EPS = 1e-5
N_CORES = 8
B_FULL = 65536
BC = B_FULL // N_CORES   # 8192 rows per core
NB = 512                 # batch columns per on-chip tile
SQD = float(1.0 / np.sqrt(D))

_NC_CACHE = {}
LAST_RESULTS = None      # BassKernelResults of the most recent kernel() call


def _build_nc(bc, nb, flags):
    use_c_dm, use_c_md, use_b1, use_b2, use_affine = flags
    nt = bc // nb
    assert nt >= 4
    nc = bacc.Bacc("TRN2", target_bir_lowering=False, debug=False,
                   num_devices=N_CORES)

    xd_d = nc.dram_tensor("xd", [D, bc], BF16, kind="ExternalInput")
    xm_d = nc.dram_tensor("xm", [D, bc], BF16, kind="ExternalInput")
    a_dm_d = nc.dram_tensor("a_dm", [D, D], BF16, kind="ExternalInput")
    a_md_d = nc.dram_tensor("a_md", [D, D], BF16, kind="ExternalInput")
    w1_d = nc.dram_tensor("w1", [DH, DF], BF16, kind="ExternalInput")
    w2_d = nc.dram_tensor("w2", [DF, D], BF16, kind="ExternalInput")
    c_dm_d = nc.dram_tensor("c_dm", [D], F32, kind="ExternalInput") if use_c_dm else None
    c_md_d = nc.dram_tensor("c_md", [D], F32, kind="ExternalInput") if use_c_md else None
    b1_d = nc.dram_tensor("b1", [DF], F32, kind="ExternalInput") if use_b1 else None
    b2_d = nc.dram_tensor("b2", [D], F32, kind="ExternalInput") if use_b2 else None
    g_o_d = nc.dram_tensor("g_o", [D], F32, kind="ExternalInput") if use_affine else None
    b_o_d = nc.dram_tensor("b_o", [D], F32, kind="ExternalInput") if use_affine else None
    o_d = nc.dram_tensor("o", [D, bc], F32, kind="ExternalOutput")

    xd_r = xd_d.ap().rearrange("(k p) n -> p k n", p=P)
    xm_r = xm_d.ap().rearrange("(k p) n -> p k n", p=P)
    o_r = o_d.ap().rearrange("(k p) n -> p k n", p=P)

    with tile.TileContext(nc) as tc, ExitStack() as ctx:
        wp = ctx.enter_context(tc.tile_pool(name="wts", bufs=1))
        xp = ctx.enter_context(tc.tile_pool(name="x", bufs=3))
        up = ctx.enter_context(tc.tile_pool(name="u", bufs=5))
        sqp = ctx.enter_context(tc.tile_pool(name="sq", bufs=3))
        xhp = ctx.enter_context(tc.tile_pool(name="xh", bufs=3))
        h1p = ctx.enter_context(tc.tile_pool(name="h1", bufs=2))
        h2p = ctx.enter_context(tc.tile_pool(name="h2", bufs=3))
        sqo = ctx.enter_context(tc.tile_pool(name="sqo", bufs=2))
        op_ = ctx.enter_context(tc.tile_pool(name="o", bufs=2))
        stp = ctx.enter_context(tc.tile_pool(name="st", bufs=1))
        bcp = ctx.enter_context(tc.tile_pool(name="bc", bufs=4))
        obcp = ctx.enter_context(tc.tile_pool(name="obc", bufs=2))
        pmm = ctx.enter_context(tc.tile_pool(name="pmm", bufs=4, space="PSUM"))
        pstat = ctx.enter_context(tc.tile_pool(name="pstat", bufs=1, space="PSUM"))

        a_dm_sb = wp.tile([P, KD, D], BF16)
        nc.gpsimd.dma_start(a_dm_sb[:], a_dm_d.ap().rearrange("(k p) m -> p k m", p=P))
        a_md_sb = wp.tile([P, KD, D], BF16)
        nc.gpsimd.dma_start(a_md_sb[:], a_md_d.ap().rearrange("(k p) m -> p k m", p=P))
        w1_sb = wp.tile([P, KH, DF], BF16)
        w2_sb = wp.tile([P, KF, D], BF16)

        # lhsT constants for the packed stat sums:
        #   cwm[:, s, :] : [P, 2] column s = 1/D      -> mean rows
        #   cws[:, s, :] : [P, 2] column s = 1.0      -> sumsq rows
        cwm = wp.tile([P, 2, 2], BF16)
        nc.vector.memset(cwm[:], 0.0)
        nc.vector.memset(cwm[:, 0, 0:1], 1.0 / D)
        nc.vector.memset(cwm[:, 1, 1:2], 1.0 / D)
        cws = wp.tile([P, 2, 2], BF16)
        nc.vector.memset(cws[:], 0.0)
        nc.vector.memset(cws[:, 0, 0:1], 1.0)
        nc.vector.memset(cws[:, 1, 1:2], 1.0)
        eps2 = wp.tile([2, 1], F32)
        nc.vector.memset(eps2[:], EPS)

        def vec_const(dram, nk, tag):
            t = wp.tile([P, nk], F32, tag=tag)
            nc.gpsimd.dma_start(t[:], dram.ap().rearrange("(k p) -> p k", p=P))
            return t

        c_dm_sb = vec_const(c_dm_d, KD, "c_dm") if use_c_dm else None
        c_md_sb = vec_const(c_md_d, KD, "c_md") if use_c_md else None
        b1_sb = vec_const(b1_d, KF, "b1") if use_b1 else None
        b2_sb = vec_const(b2_d, KD, "b2") if use_b2 else None
        g_o_sb = vec_const(g_o_d, KD, "g_o") if use_affine else None
        b_o_sb = vec_const(b_o_d, KD, "b_o") if use_affine else None

        st = [dict() for _ in range(nt)]

        def load(i):
            s = st[i]
            sl = slice(i * nb, (i + 1) * nb)
            s["xd"] = xp.tile([P, KD, nb], BF16, tag="xd", name="xdt")
            nc.sync.dma_start(s["xd"][:], xd_r[:, :, sl])
            s["xm"] = xp.tile([P, KD, nb], BF16, tag="xm", name="xmt")
            nc.sync.dma_start(s["xm"][:], xm_r[:, :, sl])

        def attn(i):
            s = st[i]

            def side(a_sb, rhs, res, c_sb, tag):
                v = up.tile([P, KD, nb], BF16, tag=tag)
                for m in range(KD):
                    ps = pmm.tile([P, nb], F32, tag="mm")
                    for k in range(KD):
                        nc.tensor.matmul(ps[:], a_sb[:, k, ts(m, P)], rhs[:, k, :],
                                         start=(k == 0), stop=(k == KD - 1))
                    nc.vector.tensor_add(v[:, m, :], ps[:], res[:, m, :])
                    if c_sb is not None:
                        nc.vector.tensor_scalar_add(v[:, m, :], v[:, m, :],
                                                    c_sb[:, m:m + 1])
                return v

            s["u"] = side(a_dm_sb, s["xm"], s["xd"], c_dm_sb, "u")
            s["w"] = side(a_md_sb, s["xd"], s["xm"], c_md_sb, "w")

        def squares(i):
            s = st[i]
            s["squ"] = sqp.tile([P, KD, nb], BF16, tag="squ", name="squ")
            s["sqw"] = sqp.tile([P, KD, nb], BF16, tag="sqw", name="sqw")
            for k in range(KD):
                nc.scalar.activation(s["squ"][:, k, :], s["u"][:, k, :],
                                     ACT.Square, scale=SQD)
                nc.scalar.activation(s["sqw"][:, k, :], s["w"][:, k, :],
                                     ACT.Square, scale=SQD)

        def sums(i):
            """ps_mu rows [mu_u, mu_w]; ps_sq rows [E[u^2], E[w^2]]."""
            s = st[i]
            ps_mu = pstat.tile([2, nb], F32, tag="smu")
            n = 0
            for sdx, x in ((0, s["u"]), (1, s["w"])):
                for k in range(KD):
                    nc.tensor.matmul(ps_mu[:], cwm[:, sdx, :], x[:, k, :],
                                     start=(n == 0), stop=(n == 2 * KD - 1))
                    n += 1
            ps_sq = pstat.tile([2, nb], F32, tag="ssq")
            n = 0
            for sdx, x in ((0, s["squ"]), (1, s["sqw"])):
                for k in range(KD):
                    nc.tensor.matmul(ps_sq[:], cws[:, sdx, :], x[:, k, :],
                                     start=(n == 0), stop=(n == 2 * KD - 1))
                    n += 1
            s["ps_mu"], s["ps_sq"] = ps_mu, ps_sq

        def chain(i):
            """stats chain on partitions 0..1; broadcast via GpSimd."""
            s = st[i]
            ps_mu, ps_sq = s.pop("ps_mu"), s.pop("ps_sq")
            mu_sb = stp.tile([2, nb], F32, tag="mu_sb")
            nc.vector.tensor_copy(mu_sb[:], ps_mu[:])
            tmp = stp.tile([2, nb], F32, tag="tmp")
            nc.vector.tensor_mul(tmp[:], mu_sb[:], mu_sb[:])
            nc.vector.tensor_sub(tmp[:], ps_sq[:], tmp[:])
            nc.scalar.activation(tmp[:], tmp[:], ACT.Sqrt, bias=eps2[:])
            inv = stp.tile([2, nb], F32, tag="inv")
            nc.vector.reciprocal_approx_fast(inv[:], tmp[:])
            s2t = stp.tile([2, 2, nb], BF16, tag="s2t")
            nc.vector.tensor_copy(s2t[:, 0, :], mu_sb[:])
            nc.vector.tensor_copy(s2t[:, 1, :], inv[:])
            # GpSimd ops must start at partition 0; DMA row 1 down first.
            s2tw = stp.tile([1, 2, nb], BF16, tag="s2tw")
            nc.sync.dma_start(s2tw[:], s2t[1:2, :, :])
            s["bcu"] = bcp.tile([P, 2, nb], BF16, tag="bc", name="bcu")
            nc.gpsimd.partition_broadcast(s["bcu"][:], s2t[0:1, :, :])
            s["bcw"] = bcp.tile([P, 2, nb], BF16, tag="bc", name="bcw")
            nc.gpsimd.partition_broadcast(s["bcw"][:], s2tw[:])

        def applies(i):
            s = st[i]
            s["xhu"] = xhp.tile([P, KD, nb], BF16, tag="xhu", name="xhu")
            s["xhw"] = xhp.tile([P, KD, nb], BF16, tag="xhw", name="xhw")
            for x, bc, xh in ((s["u"], s["bcu"], s["xhu"]),
                              (s["w"], s["bcw"], s["xhw"])):
                for k in range(KD):
                    nc.vector.tensor_sub(xh[:, k, :], x[:, k, :], bc[:, 0, :])
                    nc.vector.tensor_mul(xh[:, k, :], xh[:, k, :], bc[:, 1, :])

        def ffn1(i):
            s = st[i]
            xhu, xhw = s["xhu"], s["xhw"]
            h1 = h1p.tile([P, KF, nb], BF16, tag="h1")
            for m in range(KF):
                ps = pmm.tile([P, nb], F32, tag="mm")
                for k in range(KH):
                    rhs = xhu[:, k, :] if k < KD else xhw[:, k - KD, :]
                    nc.tensor.matmul(ps[:], w1_sb[:, k, ts(m, P)], rhs,
                                     start=(k == 0), stop=(k == KH - 1))
                if use_b1:
                    nc.scalar.activation(h1[:, m, :], ps[:], ACT.Gelu,
                                         bias=b1_sb[:, m:m + 1])
                else:
                    nc.scalar.activation(h1[:, m, :], ps[:], ACT.Gelu)
            s["h1"] = h1

        def ffn2(i):
            s = st[i]
            h1 = s["h1"]
            h2 = h2p.tile([P, KD, nb], BF16, tag="h2")
            for m in range(KD):
                ps = pmm.tile([P, nb], F32, tag="mm")
                for k in range(KF):
                    nc.tensor.matmul(ps[:], w2_sb[:, k, ts(m, P)], h1[:, k, :],
                                     start=(k == 0), stop=(k == KF - 1))
                if use_b2:
                    nc.scalar.activation(h2[:, m, :], ps[:], ACT.Identity,
                                         bias=b2_sb[:, m:m + 1])
                else:
                    nc.scalar.activation(h2[:, m, :], ps[:], ACT.Copy)
            s["h2"] = h2

        def squares_out(i):
            s = st[i]
            s["sqo"] = sqo.tile([P, KD, nb], BF16, tag="sqo", name="sqot")
            for k in range(KD):
                nc.scalar.activation(s["sqo"][:, k, :], s["h2"][:, k, :],
                                     ACT.Square, scale=SQD)

        def sums_out(i):
            s = st[i]
            ps_mu = pstat.tile([1, nb], F32, tag="somu")
            for k in range(KD):
                nc.tensor.matmul(ps_mu[:], cwm[:, 0, 0:1], s["h2"][:, k, :],
                                 start=(k == 0), stop=(k == KD - 1))
            ps_sq = pstat.tile([1, nb], F32, tag="sosq")
            for k in range(KD):
                nc.tensor.matmul(ps_sq[:], cws[:, 0, 0:1], s["sqo"][:, k, :],
                                 start=(k == 0), stop=(k == KD - 1))
            s["ps_mu_o"], s["ps_sq_o"] = ps_mu, ps_sq

        def chain_out(i):
            s = st[i]
            ps_mu, ps_sq = s.pop("ps_mu_o"), s.pop("ps_sq_o")
            mu_sb = stp.tile([1, nb], F32, tag="omu_sb")
            nc.vector.tensor_copy(mu_sb[:], ps_mu[:])
            tmp = stp.tile([1, nb], F32, tag="otmp")
            nc.vector.tensor_mul(tmp[:], mu_sb[:], mu_sb[:])
            nc.vector.tensor_sub(tmp[:], ps_sq[:], tmp[:])
            nc.scalar.activation(tmp[:], tmp[:], ACT.Sqrt, bias=eps2[0:1, :])
            inv = stp.tile([1, nb], F32, tag="oinv")
            nc.vector.reciprocal_approx_fast(inv[:], tmp[:])
            s2t = stp.tile([1, 2, nb], BF16, tag="os2t")
            nc.vector.tensor_copy(s2t[:, 0, :], mu_sb[:])
            nc.vector.tensor_copy(s2t[:, 1, :], inv[:])
            s["obc"] = obcp.tile([P, 2, nb], BF16, tag="obc", name="obct")
            nc.gpsimd.partition_broadcast(s["obc"][:], s2t[:])

        def out_apply_store(i):
            s = st[i]
            sl = slice(i * nb, (i + 1) * nb)
            h2, obc = s["h2"], s["obc"]
            o = op_.tile([P, KD, nb], F32, tag="o")
            for k in range(KD):
                nc.vector.tensor_sub(o[:, k, :], h2[:, k, :], obc[:, 0, :])
                nc.vector.tensor_mul(o[:, k, :], o[:, k, :], obc[:, 1, :])
                if use_affine:
                    nc.vector.tensor_scalar(o[:, k, :], o[:, k, :],
                                            g_o_sb[:, k:k + 1],
                                            b_o_sb[:, k:k + 1],
                                            mybir.AluOpType.mult,
                                            mybir.AluOpType.add)
                nc.sync.dma_start(o_r[:, k, sl], o[:, k, :])

        # --- software pipeline, attention 2 iters ahead ---
        # iter i: attn for tile i+2, stats for tile i, FFN for tile i-2,
        # out-LN for tile i-3.  Deep attn keeps the PE dense (and the HAM
        # clock warm) while tile 0's serial LN chain resolves.
        for t in range(min(3, nt)):
            load(t)
        for t in range(min(2, nt)):
            attn(t)
            squares(t)
        # FFN weights are not needed until ffn1(0) (~25us in); load them
        # after the early x tiles so they don't starve the attn ramp.
        nc.gpsimd.dma_start(w1_sb[:], w1_d.ap().rearrange("(k p) m -> p k m", p=P))
        nc.gpsimd.dma_start(w2_sb[:], w2_d.ap().rearrange("(k p) m -> p k m", p=P))
        for i in range(nt + 3):
            if i + 3 < nt:
                load(i + 3)
            if i + 2 < nt:
                attn(i + 2)
                squares(i + 2)
            j = i - 3
            if j >= 0:
                sums_out(j)
                chain_out(j)
            f = i - 2
            if 0 <= f < nt:
                ffn1(f)
            if i < nt:
                sums(i)
                chain(i)
            if 0 <= f < nt:
                ffn2(f)
                squares_out(f)
            if 0 <= i - 1 < nt:
                applies(i - 1)
            if j >= 0:
                out_apply_store(j)

    nc.compile()
    return nc


def kernel(**inputs) -> np.ndarray:
    global LAST_RESULTS
    f = lambda k: np.asarray(inputs[k], np.float32)

    drug = f("drug_emb")
    micro = f("micro_emb")
    b = drug.shape[0]
    bc = b // N_CORES
    assert b % (N_CORES * NB) == 0

    # ---- host-side weight folding ----
    wv_dm, bv_dm = f("dm_in_w")[2 * D:], f("dm_in_b")[2 * D:]
    wv_md, bv_md = f("md_in_w")[2 * D:], f("md_in_b")[2 * D:]
    a_dm = np.ascontiguousarray(wv_dm.T @ f("dm_out_w").T).astype(ml_dtypes.bfloat16)
    c_dm = bv_dm @ f("dm_out_w").T + f("dm_out_b")
    a_md = np.ascontiguousarray(wv_md.T @ f("md_out_w").T).astype(ml_dtypes.bfloat16)
    c_md = bv_md @ f("md_out_w").T + f("md_out_b")
    g_cat = np.concatenate([f("norm_d_g"), f("norm_m_g")])
    b_cat = np.concatenate([f("norm_d_b"), f("norm_m_b")])
    w1f = np.ascontiguousarray((f("ffn_w1") * g_cat[None, :]).T).astype(ml_dtypes.bfloat16)
    b1f = f("ffn_b1") + b_cat @ f("ffn_w1").T
    w2f = np.ascontiguousarray(f("ffn_w2").T).astype(ml_dtypes.bfloat16)
    b2 = f("ffn_b2")
    g_o, b_o = f("norm_out_g"), f("norm_out_b")

    flags = (bool(np.any(c_dm)), bool(np.any(c_md)), bool(np.any(b1f)),
             bool(np.any(b2)), bool(np.any(g_o != 1.0) or np.any(b_o)))

    key = (bc, NB, flags)
    if key not in _NC_CACHE:
        _NC_CACHE[key] = _build_nc(bc, NB, flags)
    nc = _NC_CACHE[key]

    in_maps = []
    for c in range(N_CORES):
        sl = slice(c * bc, (c + 1) * bc)
        m = {
            "xd": np.ascontiguousarray(drug[sl].T).astype(ml_dtypes.bfloat16),
            "xm": np.ascontiguousarray(micro[sl].T).astype(ml_dtypes.bfloat16),
            "a_dm": a_dm, "a_md": a_md, "w1": w1f, "w2": w2f,
        }
        if flags[0]:
            m["c_dm"] = c_dm
        if flags[1]:
            m["c_md"] = c_md
        if flags[2]:
            m["b1"] = b1f
        if flags[3]:
            m["b2"] = b2
        if flags[4]:
            m["g_o"] = g_o
            m["b_o"] = b_o
        in_maps.append(m)

    res = run_bass_kernel_spmd(nc, in_maps, list(range(N_CORES)))
    LAST_RESULTS = res

    out = np.empty((b, D), np.float32)
    for c in range(N_CORES):
        out[c * bc:(c + 1) * bc] = res.results[c]["o"].T
    return out
